# revision 18
# baseline (speedup 1.0000x reference)
"""Trainium2 Bass kernel for nn_Decoder_GCNMOE (GRU decoder + dense-blend MoE).

Strategy (8 NeuronCores, pure data parallel over batch; 16 rows/core):
  - GRU (4 layers x 384 steps): the 4 layer recurrences are interleaved at
    instruction level (layer l runs chunk w-l during wave w; within a wave
    the per-step ops of the active layers alternate) so the serial per-step
    dependency chains hide each other's latency.  Hidden state lives only as
    bf16 (feature-major); the update writes it in place.  Input projections
    (gi) for all layers (incl. layer 0, whose input sequence is precomputed
    on the host) are bulk-matmul'd per half-chunk one wave ahead.
  - MoE (6144 tokens/core): emitted as fine-grained generators drained into
    the engine idle slots left by the GRU chains.  elu is split
    relu->vector / exp->scalar / merge->STT; expert blending scales layer
    inputs by gate weights and accumulates all experts into one PSUM bank.

Assumes (guaranteed by the input spec): mask all ones, lengths==T, biases
and betas exactly zero (asserted on host).
"""

import numpy as np

# ---------------------------------------------------------------- constants
BS, T, D, H, NCLS, E, NJ, NF = 128, 384, 256, 256, 12, 4, 24, 6
MOE_H = 512
OUTD = NJ * NF            # 144
NCORES = 8
B = BS // NCORES          # 16 batch rows per core
KT = H // 128             # 2 k-tiles over H
OT = 3 * H // 128         # 6 o-tiles over 3H
L = 4                     # GRU layers
CH = 16                   # chunk length in steps
TOKC = 512                # MoE tokens per chunk

_STATE = {}


def _bf16(x):
    import ml_dtypes
    return np.asarray(x, dtype=ml_dtypes.bfloat16)


def _f32(x):
    return np.ascontiguousarray(np.asarray(x, dtype=np.float32))


# ------------------------------------------------------------ device program
def _build_nc(T_=T, debug=False):
    import concourse.bass as bass
    import concourse.mybir as mybir
    import concourse.tile as tile

    f32, bf16 = mybir.dt.float32, mybir.dt.bfloat16
    AF, ALU = mybir.ActivationFunctionType, mybir.AluOpType
    NCH = T_ // CH
    NTOK = B * T_
    NMC = NTOK // TOKC

    nc = bass.Bass()
    dt_in = {}

    def din(name, shape, dt=bf16):
        dt_in[name] = nc.dram_tensor(name, list(shape), dt, kind="ExternalInput")
        return dt_in[name]

    whh_t = din("whh_t", [128, L * KT * OT * 128])
    wih_t = din("wih_t", [128, L * KT * OT * 128])
    x_all_d = din("x_all", [128, KT * NTOK])
    g0t = din("g0t", [128, KT * 4 * 128])
    g1t = din("g1t", [128, 4 * 4 * 128])
    g2r = din("g2r", [128, 4 * 4])
    a0t = din("a0t", [128, E * 2 * 4 * 128])
    a1t = din("a1t", [128, E * 4 * 4 * 128])
    a2ta = din("a2ta", [128, E * 4 * 128])
    a2tb = din("a2tb", [128, E * 4 * 16])
    idm = din("idm", [128, 128], f32)
    out_d = nc.dram_tensor("out", [OUTD, T_, B], f32, kind="ExternalOutput")
    if debug:
        dbg = {n: nc.dram_tensor(n, sh, dt, kind="ExternalOutput") for n, sh, dt in [
            ("dbg_zf", [128, KT * NTOK], bf16),
            ("dbg_gi", [128, 2 * CH * L * OT * B], bf16)]}

    with tile.TileContext(nc) as tc:
        with (
            tc.tile_pool(name="wpool", bufs=1) as wp,      # resident weights
            tc.tile_pool(name="state", bufs=1) as sp,      # persistent activations
            tc.tile_pool(name="work", bufs=6) as wk,       # small rotating tiles
            tc.tile_pool(name="welu", bufs=2) as we,       # elu intermediates
            tc.tile_pool(name="xch", bufs=2) as xch,       # layer-0 input chunks
            tc.tile_pool(name="moe", bufs=1) as mp,        # MoE chunk tensors
            tc.tile_pool(name="moesc", bufs=2) as msc,     # blend2 scaled input
            tc.tile_pool(name="ps_gru", bufs=1, space="PSUM") as pgru,
            tc.tile_pool(name="ps_gib", bufs=1, space="PSUM") as pgib,
            tc.tile_pool(name="ps_moe", bufs=2, space="PSUM") as pmo,
            tc.tile_pool(name="ps_sm", bufs=1, space="PSUM") as psm,
        ):
            # ---- resident weight tiles
            def load(name, dram, shape, dt=bf16):
                t_ = wp.tile(shape, dt, tag=name)
                nc.sync.dma_start(t_[:], dram[:])
                return t_

            _whh = load("whh", whh_t, [128, L * KT * OT * 128])
            _wih = load("wih", wih_t, [128, L * KT * OT * 128])
            _g0 = load("g0", g0t, [128, KT * 4 * 128])
            _g1 = load("g1", g1t, [128, 4 * 4 * 128])
            _g2r = load("g2r", g2r, [128, 4 * 4])
            _a0 = load("a0", a0t, [128, E * 2 * 4 * 128])
            _a1 = load("a1", a1t, [128, E * 4 * 4 * 128])
            _a2a = load("a2a", a2ta, [128, E * 4 * 128])
            _a2b = load("a2b", a2tb, [128, E * 4 * 16])
            w_idm = load("idm", idm, [128, 128], f32)

            def _sl(tile_, idx, c):
                return tile_[:, idx * c:(idx + 1) * c]

            w_whh = lambda l, kt, ot: _sl(_whh, (l * KT + kt) * OT + ot, 128)
            w_wih = lambda l, kt, ot: _sl(_wih, (l * KT + kt) * OT + ot, 128)
            w_g0 = lambda kt, mt: _sl(_g0, kt * 4 + mt, 128)
            w_g1 = lambda kt, mt: _sl(_g1, kt * 4 + mt, 128)
            w_g2r = lambda kt: _sl(_g2r, kt, 4)
            w_a0 = lambda e, kt, mt: _sl(_a0, (e * 2 + kt) * 4 + mt, 128)
            w_a1 = lambda e, kt, mt: _sl(_a1, (e * 4 + kt) * 4 + mt, 128)
            w_a2a = lambda e, kt: _sl(_a2a, e * 4 + kt, 128)
            w_a2b = lambda e, kt: _sl(_a2b, e * 4 + kt, 16)

            ones1 = sp.tile([1, 128], bf16, tag="ones1")
            nc.gpsimd.memset(ones1[:], 1.0)
            zero_h = sp.tile([128, KT * B], bf16, tag="zero_h")
            nc.gpsimd.memset(zero_h[:], 0.0)

            # hidden-state history ring: rows = t % 32, layout (row, l, kt, b)
            # (layer 3's h lives in zfT instead; its hist slot stays unused).
            HROW = L * KT * B                               # 128 per row
            hist = sp.tile([128, 32 * HROW], bf16, tag="hist")
            zfT = sp.tile([128, KT * NTOK], bf16, tag="zfT")
            # gi buffer: (par, s, l, ot, b); written one wave ahead
            GROW = L * OT * B                               # 384 per step-row
            gi_all = sp.tile([128, 2 * CH * GROW], bf16, tag="gi_all")

            # layer-0 input chunks streamed from DRAM
            x_bufs = {}

            def load_x(c):
                t_ = xch.tile([128, KT * CH * B], bf16, tag="xc")
                nc.sync.dma_start(
                    t_[:].rearrange("p (k n) -> p k n", k=KT),
                    x_all_d[:].rearrange("p (k n) -> p k n", k=KT)[
                        :, :, c * CH * B:(c + 1) * CH * B])
                x_bufs[c] = t_

            # ---------------------------------------------- GRU helpers
            def h_src(l, t, kt):
                # bf16 hidden state [128, B] (step-matmul rhs)
                if t < 0:
                    return zero_h[:, kt * B:(kt + 1) * B]
                if l < 3:
                    r = (t % 32) * HROW
                    return hist[:, r + (l * KT + kt) * B: r + (l * KT + kt + 1) * B]
                return zfT[:, kt * NTOK + t * B: kt * NTOK + t * B + B]

            def h_prev3(l, t):
                # [128, kt, b] view of h(t)
                if t < 0:
                    return zero_h[:].rearrange("p (k b) -> p k b", b=B)
                if l < 3:
                    r = (t % 32) * HROW + l * KT * B
                    return hist[:, r:r + KT * B].rearrange(
                        "p (k b) -> p k b", b=B)
                return zfT[:].rearrange("p (k n) -> p k n", k=KT)[
                    :, :, t * B:(t + 1) * B]

            def gi3(l, t, o0, o1):
                # [128, o, b] gi slice for step t, o-tiles [o0, o1)
                par, s = (t // CH) % 2, t % CH
                base = (par * CH + s) * GROW + (l * OT + o0) * B
                return gi_all[:, base:base + (o1 - o0) * B].rearrange(
                    "p (o b) -> p o b", b=B)

            def gin_rhs(l, c, r0, r1, kt):
                # bulk-gi rhs rows [r0, r1) of chunk c of layer l's input
                if l == 0:
                    return x_bufs[c][:, kt * CH * B + r0 * B:
                                     kt * CH * B + r1 * B]
                s0 = (c * CH) % 32
                return hist[:].rearrange("p (s z) -> p s z", z=HROW)[
                    :, s0 + r0:s0 + r1,
                    ((l - 1) * KT + kt) * B:((l - 1) * KT + kt + 1) * B]

            # manually-sliced PSUM rings (bank-packed: 4 GRU step slots and
            # 2 bulk-gi slots share one bank each)
            HCH = CH // 2
            gh_ring = pgru.tile([128, 4 * OT * B], f32, tag="ghring")
            gib_ring = pgib.tile([128, 2 * HCH * B], f32, tag="gibring")
            rc = {"gh": 0, "gib": 0}

            # bulk gi: one (layer, chunk, ot, half) unit = 2 matmuls + 1 copy
            def emit_gi_unit(l, c, ot, half):
                par = c % 2
                r0 = half * HCH
                rc["gib"] += 1
                i = (rc["gib"] % 2) * HCH * B
                pb = gib_ring[:, i:i + HCH * B]
                for kt in range(KT):
                    nc.tensor.matmul(pb, w_wih(l, kt, ot),
                                     gin_rhs(l, c, r0, r0 + HCH, kt),
                                     start=(kt == 0), stop=(kt == KT - 1))
                dst = gi_all[:].rearrange("p (s z) -> p s z", z=GROW)[
                    :, par * CH + r0:par * CH + r0 + HCH,
                    (l * OT + ot) * B:(l * OT + ot + 1) * B]
                pb3 = pb.rearrange("p (s b) -> p s b", b=B)
                nc.vector.tensor_copy(dst, pb3)

            # one GRU step for layer l at absolute time t
            def emit_step(l, t):
                rc["gh"] += 1
                i0 = (rc["gh"] % 4) * OT * B
                pgs = lambda o0, o1: gh_ring[:, i0 + o0 * B:i0 + o1 * B]
                for ot in range(OT):
                    for kt in range(KT):
                        nc.tensor.matmul(
                            pgs(ot, ot + 1),
                            w_whh(l, kt, ot), h_src(l, t - 1, kt),
                            start=(kt == 0), stop=(kt == KT - 1))
                p3 = lambda a: a.rearrange("p (o b) -> p o b", b=B)
                # rz gates
                s_in = wk.tile([128, 4 * B], f32, tag="s_in")
                nc.vector.tensor_tensor(p3(s_in[:]), p3(pgs(0, 4)),
                                        gi3(l, t, 0, 4), op=ALU.add)
                ru = wk.tile([128, 4 * B], f32, tag="ru")
                nc.scalar.activation(ru[:], s_in[:], AF.Sigmoid)
                # n gate: nt = tanh(r * psum_n + gi_n)
                hn = wk.tile([128, 2 * B], f32, tag="hn")
                nc.vector.tensor_tensor(hn[:], ru[:, 0:2 * B],
                                        pgs(4, 6), op=ALU.mult)
                n_in = wk.tile([128, 2 * B], f32, tag="n_in")
                nc.gpsimd.tensor_tensor(p3(n_in[:]), p3(hn[:]),
                                        gi3(l, t, 4, 6), op=ALU.add)
                nt = wk.tile([128, 2 * B], f32, tag="nt")
                nc.scalar.activation(nt[:], n_in[:], AF.Tanh)
                # h' = nt + u * (h - nt)
                dt_ = wk.tile([128, 2 * B], f32, tag="dt")
                nc.gpsimd.tensor_tensor(p3(dt_[:]), h_prev3(l, t - 1),
                                        p3(nt[:]), op=ALU.subtract)
                mt_ = wk.tile([128, 2 * B], f32, tag="mt")
                nc.vector.tensor_tensor(mt_[:], ru[:, 2 * B:4 * B], dt_[:],
                                        op=ALU.mult)
                nc.vector.tensor_tensor(h_prev3(l, t), p3(nt[:]), p3(mt_[:]),
                                        op=ALU.add)

            # ---------------------------------------------- MoE generator
            def elu_emit(dst, src_ps, merge_eng):
                # dst(bf16) = elu(src) = (relu(x) - 1) + min(exp(x), 1)
                tr = we.tile([128, TOKC], f32, tag="elu_r")
                nc.vector.tensor_scalar(tr[:], src_ps, 0.0, -1.0,
                                        op0=ALU.max, op1=ALU.add)
                te = we.tile([128, TOKC], f32, tag="elu_e")
                nc.scalar.activation(te[:], src_ps, AF.Exp)
                nc.vector.scalar_tensor_tensor(dst, te[:], 1.0, tr[:],
                                               op0=ALU.min, op1=ALU.add)

            def moe_gen(ctk):
                def zc(kt):
                    return zfT[:, kt * NTOK + ctk * TOKC:
                               kt * NTOK + (ctk + 1) * TOKC]

                # gating layer 0
                g0o = mp.tile([128, 4 * TOKC], bf16, tag="g0o")
                for mt in range(4):
                    pb = pmo.tile([128, TOKC], f32, tag="mo")
                    for kt in range(KT):
                        nc.tensor.matmul(pb[:], w_g0(kt, mt), zc(kt),
                                         start=(kt == 0), stop=(kt == KT - 1))
                    elu_emit(g0o[:, mt * TOKC:(mt + 1) * TOKC], pb[:],
                             nc.gpsimd)
                    yield
                # gating layer 1
                g1o = mp.tile([128, 4 * TOKC], bf16, tag="g1o")
                for mt in range(4):
                    pb = pmo.tile([128, TOKC], f32, tag="mo")
                    for kt in range(4):
                        nc.tensor.matmul(pb[:], w_g1(kt, mt),
                                         g0o[:, kt * TOKC:(kt + 1) * TOKC],
                                         start=(kt == 0), stop=(kt == 3))
                    elu_emit(g1o[:, mt * TOKC:(mt + 1) * TOKC], pb[:],
                             nc.gpsimd)
                    yield
                # gating layer 2 (token-major) + softmax
                pg2 = psm.tile([128, 16], f32, tag="pg2")
                for tt in range(4):
                    for kt in range(4):
                        nc.tensor.matmul(
                            pg2[:, tt * 4:(tt + 1) * 4],
                            g1o[:, kt * TOKC + tt * 128:
                                kt * TOKC + (tt + 1) * 128],
                            w_g2r(kt), start=(kt == 0), stop=(kt == 3),
                            skip_group_check=True)
                yield
                g4 = lambda a: a.rearrange("p (g e) -> p g e", e=4)
                mx = wk.tile([128, 4], f32, tag="sm_mx")
                nc.vector.tensor_reduce(mx[:], g4(pg2[:]),
                                        axis=mybir.AxisListType.X, op=ALU.max)
                sub = wk.tile([128, 16], f32, tag="sm_sub")
                nc.vector.tensor_tensor(g4(sub[:]), g4(pg2[:]),
                                        mx[:].unsqueeze(2).broadcast_to(
                                            (128, 4, 4)), op=ALU.subtract)
                ex = wk.tile([128, 16], f32, tag="sm_ex")
                nc.scalar.activation(ex[:], sub[:], AF.Exp)
                sm = wk.tile([128, 4], f32, tag="sm_sum")
                nc.vector.tensor_reduce(sm[:], g4(ex[:]),
                                        axis=mybir.AxisListType.X, op=ALU.add)
                rec = wk.tile([128, 4], f32, tag="sm_rec")
                nc.vector.reciprocal(rec[:], sm[:])
                gw = wk.tile([128, 16], f32, tag="sm_gw")
                nc.vector.tensor_tensor(g4(gw[:]), g4(ex[:]),
                                        rec[:].unsqueeze(2).broadcast_to(
                                            (128, 4, 4)), op=ALU.mult)
                yield
                # transpose gw -> [4, 512], broadcast to [128, 512] per e
                pgt = psm.tile([4, TOKC], f32, tag="pgt")
                for tt in range(4):
                    nc.tensor.transpose(pgt[:, tt * 128:(tt + 1) * 128],
                                        gw[:, tt * 4:(tt + 1) * 4], w_idm[:])
                gwT = mp.tile([4, TOKC], bf16, tag="gwT")
                nc.vector.tensor_copy(gwT[:], pgt[:])
                gwf = mp.tile([1, E * TOKC], bf16, tag="gwf")
                nc.sync.dma_start(
                    gwf[:].rearrange("p (e n) -> p e n", e=E), gwT[:])
                gwb = mp.tile([128, E * TOKC], bf16, tag="gwb")
                for e in range(E):
                    pbc = pmo.tile([128, TOKC], f32, tag="mo")
                    nc.tensor.matmul(pbc[:], ones1[:],
                                     gwf[0:1, e * TOKC:(e + 1) * TOKC],
                                     start=True, stop=True)
                    nc.vector.tensor_copy(gwb[:, e * TOKC:(e + 1) * TOKC],
                                          pbc[:])
                    yield

                def gwb_e(e):
                    return gwb[:, e * TOKC:(e + 1) * TOKC]

                # blend 0: scale zf by gates (8 tiles), then mt-outer matmuls
                zsc = mp.tile([128, E * KT * TOKC], bf16, tag="zsc")
                for e in range(E):
                    for kt in range(KT):
                        nc.vector.tensor_tensor(
                            _sl(zsc, e * KT + kt, TOKC), zc(kt), gwb_e(e),
                            op=ALU.mult)
                    yield
                h1o = mp.tile([128, 4 * TOKC], bf16, tag="h1o")
                for mt in range(4):
                    pb = pmo.tile([128, TOKC], f32, tag="mo")
                    for e in range(E):
                        for kt in range(KT):
                            nc.tensor.matmul(
                                pb[:], w_a0(e, kt, mt),
                                _sl(zsc, e * KT + kt, TOKC),
                                start=(e == 0 and kt == 0),
                                stop=(e == E - 1 and kt == KT - 1),
                                skip_group_check=True)
                    elu_emit(h1o[:, mt * TOKC:(mt + 1) * TOKC], pb[:],
                             nc.vector)
                    yield
                # blend 1: scale h1o (16 tiles), mt-outer
                h1sc = mp.tile([128, E * 4 * TOKC], bf16, tag="h1sc")
                for e in range(E):
                    for kt in range(4):
                        nc.vector.tensor_tensor(
                            _sl(h1sc, e * 4 + kt, TOKC),
                            h1o[:, kt * TOKC:(kt + 1) * TOKC], gwb_e(e),
                            op=ALU.mult)
                    yield
                h2o = mp.tile([128, 4 * TOKC], bf16, tag="h2o")
                for mt in range(4):
                    pb = pmo.tile([128, TOKC], f32, tag="mo")
                    for e in range(E):
                        for kt in range(4):
                            nc.tensor.matmul(
                                pb[:], w_a1(e, kt, mt),
                                _sl(h1sc, e * 4 + kt, TOKC),
                                start=(e == 0 and kt == 0),
                                stop=(e == E - 1 and kt == 3),
                                skip_group_check=True)
                    elu_emit(h2o[:, mt * TOKC:(mt + 1) * TOKC], pb[:],
                             nc.vector)
                    yield
                # blend 2: out 144 = 128 + 16 (e-outer, scaled input per e)
                poa = pmo.tile([128, TOKC], f32, tag="mo")
                pob = psm.tile([16, TOKC], f32, tag="pob")
                for e in range(E):
                    h2sc = msc.tile([128, 4 * TOKC], bf16, tag="h2sc")
                    for kt in range(4):
                        nc.vector.tensor_tensor(
                            _sl(h2sc, kt, TOKC),
                            h2o[:, kt * TOKC:(kt + 1) * TOKC], gwb_e(e),
                            op=ALU.mult)
                    yield
                    for kt in range(4):
                        last = (e == E - 1 and kt == 3)
                        nc.tensor.matmul(poa[:], w_a2a(e, kt),
                                         _sl(h2sc, kt, TOKC),
                                         start=(e == 0 and kt == 0),
                                         stop=last, skip_group_check=True)
                        nc.tensor.matmul(pob[:], w_a2b(e, kt),
                                         _sl(h2sc, kt, TOKC),
                                         start=(e == 0 and kt == 0),
                                         stop=last, skip_group_check=True)
                    yield
                oa = mp.tile([128, TOKC], f32, tag="oa")
                nc.vector.tensor_copy(oa[:], poa[:])
                ob = mp.tile([16, TOKC], f32, tag="ob")
                nc.vector.tensor_copy(ob[:], pob[:])
                t0 = ctk * (TOKC // B)
                nc.sync.dma_start(out_d[0:128, t0:t0 + 32, :],
                                  oa[:].rearrange("p (t b) -> p t b", b=B))
                nc.sync.dma_start(out_d[128:144, t0:t0 + 32, :],
                                  ob[:].rearrange("p (t b) -> p t b", b=B))
                yield

            # ---------------------------------------------- main schedule
            NW = NCH + L - 1
            moe_ready = {2 * c + 5: c for c in range(NMC)}
            moe_active = []

            def drain_moe(n):
                while n > 0 and moe_active:
                    try:
                        next(moe_active[0])
                        n -= 1
                    except StopIteration:
                        moe_active.pop(0)

            load_x(0)                 # layer-0 chunk 0 input + gi before wave 0
            for ot in range(OT):
                for hf in range(2):
                    emit_gi_unit(0, 0, ot, hf)
            for w in range(NW):
                active = [l for l in range(L) if 0 <= w - l < NCH]
                nxt = [l for l in range(L) if 0 <= w + 1 - l < NCH]
                if w in moe_ready:
                    moe_active.append(moe_gen(moe_ready[w]))
                gi_early = ([(0, w + 1, ot, hf) for ot in range(OT)
                             for hf in range(2)] if (0 in nxt and w + 1 < NCH)
                            else [])
                if gi_early:
                    load_x(w + 1)
                gi_h0 = [(l, w + 1 - l, ot, 0) for l in nxt if l >= 1
                         for ot in range(OT)]
                gi_h1 = [(l, w + 1 - l, ot, 1) for l in nxt if l >= 1
                         for ot in range(OT)]
                for s in range(CH):
                    for l in active:
                        emit_step(l, (w - l) * CH + s)
                    if s < 6:
                        for u in gi_early[2 * s:2 * s + 2]:
                            emit_gi_unit(*u)
                    if s >= 8:
                        k = (s - 8) * 3
                        for u in gi_h0[k:k + 3]:
                            emit_gi_unit(*u)
                    drain_moe(2)
                for u in gi_h1:
                    emit_gi_unit(*u)
            for w_, c_ in sorted(moe_ready.items()):
                if w_ >= NW:          # chunks whose ready-wave is past the end
                    moe_active.append(moe_gen(c_))
            while moe_active:
                drain_moe(1000)

            if debug:
                nc.sync.dma_start(dbg["dbg_zf"][:], zfT[:])
                nc.sync.dma_start(dbg["dbg_gi"][:], gi_all[:])
    return nc


# ------------------------------------------------------------- walrus fixup
def _fix_sync_waits(nc, max_waits=1):
    """This walrus build allows only one sync wait per instruction; move
    excess waits onto NOPs inserted ahead of the instruction."""
    import concourse.mybir as mybir
    import bass_rust
    ctr = 0
    for f in nc.m.functions:
        for blk in f.blocks:
            out = []
            changed = False
            for inst in blk.instructions:
                si = inst.sync_info
                if si is not None and si.on_wait and len(si.on_wait) > max_waits:
                    waits = list(si.on_wait)
                    extra, keep = waits[:-max_waits], waits[-max_waits:]
                    for w_ in extra:
                        ctr += 1
                        nop = mybir.InstNoOp(name=f"WSPLIT-{ctr}", ins=[], outs=[])
                        nop.engine = inst.engine
                        nop.sync_info = bass_rust.SyncInfo(on_wait=[w_], on_update=[])
                        out.append(nop)
                    inst.sync_info = bass_rust.SyncInfo(
                        on_wait=keep, on_update=list(si.on_update))
                    changed = True
                out.append(inst)
            if changed:
                blk.instructions = out
    return ctr


# ------------------------------------------------------------- preprocessing
def _prep_core_inputs(inputs, T_=T):
    z = _f32(inputs["z"])
    y = np.asarray(inputs["y"]).astype(np.int64)
    lengths = np.asarray(inputs["lengths"]).astype(np.float64)
    emb_w = _f32(inputs["emb_w"])      # [H, D+NC+1]
    gru_wih = _f32(inputs["gru_wih"])  # [4, 3H, H]
    gru_whh = _f32(inputs["gru_whh"])
    g0_w = _f32(inputs["g0_w"]); g1_w = _f32(inputs["g1_w"]); g2_w = _f32(inputs["g2_w"])
    g0_b = _f32(inputs["g0_b"]); g1_b = _f32(inputs["g1_b"]); g2_b = _f32(inputs["g2_b"])
    a0 = _f32(inputs["alpha0"]); a1 = _f32(inputs["alpha1"]); a2 = _f32(inputs["alpha2"])
    b0 = _f32(inputs["beta0"]); b1 = _f32(inputs["beta1"]); b2 = _f32(inputs["beta2"])
    emb_b = _f32(inputs["emb_b"])
    bsum = _f32(inputs["gru_bih"]) + _f32(inputs["gru_bhh"])  # [4, 3H] assumed zero

    def pack_lhsT(w, cols=128):
        # w: [O, K]; lhsT = w.T tiled [K//128, O//cols, 128, cols]
        # -> flat [128, ntiles*cols], tile index = kt*OT_ + ot (kt-major)
        O, K = w.shape
        ktn, otn = K // 128, O // cols
        wt = np.ascontiguousarray(w.T).reshape(ktn, 128, otn, cols)
        return wt.transpose(1, 0, 2, 3).reshape(128, ktn * otn * cols)

    whh_t = _bf16(np.concatenate([pack_lhsT(gru_whh[l]) for l in range(4)], axis=1))
    wih_t = _bf16(np.concatenate([pack_lhsT(gru_wih[l]) for l in range(4)], axis=1))
    g0t = _bf16(pack_lhsT(g0_w))
    g1t = _bf16(pack_lhsT(g1_w))
    g2r = _bf16(np.ascontiguousarray(g2_w.T).reshape(4, 128, 4)
                .transpose(1, 0, 2).reshape(128, 16))
    a0t = _bf16(np.concatenate([pack_lhsT(a0[e]) for e in range(E)], axis=1))
    a1t = _bf16(np.concatenate([pack_lhsT(a1[e]) for e in range(E)], axis=1))
    a2T = np.stack([np.ascontiguousarray(a2[e].T) for e in range(E)])  # [E,512,144]
    a2r = a2T.reshape(E, 4, 128, 144)
    a2ta = _bf16(a2r[:, :, :, 0:128].transpose(2, 0, 1, 3).reshape(128, E * 4 * 128))
    a2tb = _bf16(a2r[:, :, :, 128:144].transpose(2, 0, 1, 3).reshape(128, E * 4 * 16))
    idm = _f32(np.eye(128))
    for _nm, _v in [("g0_b", g0_b), ("g1_b", g1_b), ("g2_b", g2_b),
                    ("emb_b", emb_b), ("beta0", b0), ("beta1", b1),
                    ("beta2", b2), ("gru_b", bsum)]:
        assert np.abs(_v).max() == 0.0, f"{_nm} nonzero; kernel assumes zero"

    # ---- per-core layer-0 input sequence x_all[f, (kt|t|b)]
    yoh = np.zeros((BS, NCLS), np.float32)
    yoh[np.arange(BS), y] = 1.0
    za = np.concatenate([z, yoh], axis=1)              # [BS, 268]
    u_all = za @ emb_w[:, :268].T                      # [BS, H]
    cb = (1.0 / (lengths - 1.0)).astype(np.float32)    # time scale per row
    e268 = emb_w[:, 268]                               # [H]
    tt_ = np.arange(T_, dtype=np.float32)

    maps = []
    for cidx in range(NCORES):
        sl = slice(cidx * B, (cidx + 1) * B)
        # x[f, t, b] = u[b, f] + t * cb_b * e268[f]
        xc = (u_all[sl].T[:, None, :]
              + e268[:, None, None] * (tt_[None, :, None] * cb[sl][None, None, :]))
        x_pack = _bf16(xc.reshape(KT, 128, T_ * B)
                       .transpose(1, 0, 2).reshape(128, KT * T_ * B))
        maps.append(dict(
            whh_t=whh_t, wih_t=wih_t, x_all=x_pack,
            g0t=g0t, g1t=g1t, g2r=g2r,
            a0t=a0t, a1t=a1t, a2ta=a2ta, a2tb=a2tb, idm=idm,
        ))
    return maps


# ------------------------------------------------------------------ runner
def _get_runner(T_=T):
    key = ("runner", T_)
    if key in _STATE:
        return _STATE[key]
    import jax
    from jax.sharding import Mesh, PartitionSpec
    try:
        from jax.experimental.shard_map import shard_map
    except ImportError:
        from jax.shard_map import shard_map
    import concourse.mybir as mybir
    from concourse import bass2jax

    nc = _build_nc(T_)
    _fix_sync_waits(nc)
    bass2jax.install_neuronx_cc_hook()
    partition_name = nc.partition_id_tensor.name if nc.partition_id_tensor else None
    in_names, out_names, out_avals = [], [], []
    for alloc in nc.m.functions[0].allocations:
        if not isinstance(alloc, mybir.MemoryLocationSet):
            continue
        name = alloc.memorylocations[0].name
        if alloc.kind == "ExternalInput":
            if name != partition_name:
                in_names.append(name)
        elif alloc.kind == "ExternalOutput":
            out_names.append(name)
            out_avals.append(jax.core.ShapedArray(
                tuple(alloc.tensor_shape), mybir.dt.np(alloc.dtype)))
    n_params = len(in_names)
    all_names = in_names + out_names + ([partition_name] if partition_name else [])

    def _body(*args):
        operands = list(args)
        if partition_name is not None:
            operands.append(bass2jax.partition_id_tensor())
        return tuple(bass2jax._bass_exec_p.bind(
            *operands, out_avals=tuple(out_avals), in_names=tuple(all_names),
            out_names=tuple(out_names), lowering_input_output_aliases=(),
            sim_require_finite=True, sim_require_nnan=True, nc=nc))

    devices = jax.devices()[:NCORES]
    mesh = Mesh(np.asarray(devices), ("core",))
    n_outs = len(out_names)
    sharded = jax.jit(
        shard_map(_body, mesh=mesh,
                  in_specs=(PartitionSpec("core"),) * (n_params + n_outs),
                  out_specs=(PartitionSpec("core"),) * n_outs),
        keep_unused=True)
    runner = dict(sharded=sharded, in_names=in_names, out_names=out_names,
                  out_avals=out_avals)
    _STATE[key] = runner
    return runner


def _run_device(maps, T_=T, timing=None):
    import jax
    r = _get_runner(T_)
    concat_in = [np.concatenate([np.asarray(maps[c][n]) for c in range(NCORES)],
                                axis=0) for n in r["in_names"]]
    zeros = [np.zeros((NCORES * a.shape[0], *a.shape[1:]), a.dtype)
             for a in r["out_avals"]]
    din = [jax.device_put(x) for x in concat_in]
    dz = [jax.device_put(z) for z in zeros]
    out = r["sharded"](*din, *dz)
    jax.block_until_ready(out)
    if timing is not None:
        import time
        for _ in range(timing.get("iters", 10)):
            t0 = time.perf_counter()
            out = r["sharded"](*din, *dz)
            jax.block_until_ready(out)
            timing.setdefault("times", []).append(time.perf_counter() - t0)
    o = np.asarray(out[0])
    per_core = o.reshape(NCORES, OUTD, T_, B)
    return per_core


def kernel(**inputs):
    maps = _prep_core_inputs(inputs, T)
    per_core = _run_device(maps, T)          # [NCORES, OUTD, T, B]
    full = per_core.transpose(0, 3, 1, 2)    # [NCORES, B, OUTD, T]
    full = full.reshape(BS, NJ, NF, T)
    return np.ascontiguousarray(full.astype(np.float32))


# revision 25
# speedup vs baseline: 1.0157x; 1.0157x over previous
"""Trainium2 Bass kernel for nn_Decoder_GCNMOE (GRU decoder + dense-blend MoE).

Strategy (8 NeuronCores, pure data parallel over batch; 16 rows/core):
  - GRU (4 layers x 384 steps): the 4 layer recurrences are interleaved at
    instruction level (layer l runs chunk w-l during wave w; within a wave
    the per-step ops of the active layers alternate) so the serial per-step
    dependency chains hide each other's latency.  Hidden state lives only as
    bf16 (feature-major); the update writes it in place.  Input projections
    (gi) for all layers (incl. layer 0, whose input sequence is precomputed
    on the host) are bulk-matmul'd per half-chunk one wave ahead.
  - MoE (6144 tokens/core): emitted as fine-grained generators drained into
    the engine idle slots left by the GRU chains.  elu is split
    relu->vector / exp->scalar / merge->STT; expert blending scales layer
    inputs by gate weights and accumulates all experts into one PSUM bank.

Assumes (guaranteed by the input spec): mask all ones, lengths==T, biases
and betas exactly zero (asserted on host).
"""

import numpy as np

# ---------------------------------------------------------------- constants
BS, T, D, H, NCLS, E, NJ, NF = 128, 384, 256, 256, 12, 4, 24, 6
MOE_H = 512
OUTD = NJ * NF            # 144
NCORES = 8
B = BS // NCORES          # 16 batch rows per core
KT = H // 128             # 2 k-tiles over H
OT = 3 * H // 128         # 6 o-tiles over 3H
L = 4                     # GRU layers
CH = 16                   # chunk length in steps
TOKC = 512                # MoE tokens per chunk

_STATE = {}


def _bf16(x):
    import ml_dtypes
    return np.asarray(x, dtype=ml_dtypes.bfloat16)


def _f32(x):
    return np.ascontiguousarray(np.asarray(x, dtype=np.float32))


# ------------------------------------------------------------ device program
def _build_nc(T_=T, debug=False):
    import concourse.bass as bass
    import concourse.mybir as mybir
    import concourse.tile as tile

    f32, bf16 = mybir.dt.float32, mybir.dt.bfloat16
    AF, ALU = mybir.ActivationFunctionType, mybir.AluOpType
    NCH = T_ // CH
    NTOK = B * T_
    NMC = NTOK // TOKC

    nc = bass.Bass()
    dt_in = {}

    def din(name, shape, dt=bf16):
        dt_in[name] = nc.dram_tensor(name, list(shape), dt, kind="ExternalInput")
        return dt_in[name]

    whh_t = din("whh_t", [128, L * KT * OT * 128])
    wih_t = din("wih_t", [128, L * KT * OT * 128])
    x_all_d = din("x_all", [128, KT * NTOK])
    g0t = din("g0t", [128, KT * 4 * 128])
    g1t = din("g1t", [128, 4 * 4 * 128])
    g2r = din("g2r", [128, 4 * 4])
    a0t = din("a0t", [128, E * 2 * 4 * 128])
    a1t = din("a1t", [128, E * 4 * 4 * 128])
    a2ta = din("a2ta", [128, E * 4 * 128])
    a2tb = din("a2tb", [128, E * 4 * 16])
    idm = din("idm", [128, 128], f32)
    out_d = nc.dram_tensor("out", [OUTD, T_, B], f32, kind="ExternalOutput")
    if debug:
        dbg = {n: nc.dram_tensor(n, sh, dt, kind="ExternalOutput") for n, sh, dt in [
            ("dbg_zf", [128, KT * NTOK], bf16),
            ("dbg_gi", [128, 2 * CH * L * OT * B], bf16)]}

    with tile.TileContext(nc) as tc:
        with (
            tc.tile_pool(name="wpool", bufs=1) as wp,      # resident weights
            tc.tile_pool(name="state", bufs=1) as sp,      # persistent activations
            tc.tile_pool(name="work", bufs=6) as wk,       # small rotating tiles
            tc.tile_pool(name="welu", bufs=2) as we,       # elu intermediates
            tc.tile_pool(name="xch", bufs=2) as xch,       # layer-0 input chunks
            tc.tile_pool(name="moe", bufs=1) as mp,        # MoE chunk tensors
            tc.tile_pool(name="moesc", bufs=2) as msc,     # blend2 scaled input
            tc.tile_pool(name="ps_gru", bufs=1, space="PSUM") as pgru,
            tc.tile_pool(name="ps_gib", bufs=1, space="PSUM") as pgib,
            tc.tile_pool(name="ps_moe", bufs=2, space="PSUM") as pmo,
            tc.tile_pool(name="ps_sm", bufs=1, space="PSUM") as psm,
        ):
            # ---- resident weight tiles
            def load(name, dram, shape, dt=bf16):
                t_ = wp.tile(shape, dt, tag=name)
                nc.sync.dma_start(t_[:], dram[:])
                return t_

            _whh = load("whh", whh_t, [128, L * KT * OT * 128])
            _wih = load("wih", wih_t, [128, L * KT * OT * 128])
            _g0 = load("g0", g0t, [128, KT * 4 * 128])
            _g1 = load("g1", g1t, [128, 4 * 4 * 128])
            _g2r = load("g2r", g2r, [128, 4 * 4])
            _a0 = load("a0", a0t, [128, E * 2 * 4 * 128])
            _a1 = load("a1", a1t, [128, E * 4 * 4 * 128])
            _a2a = load("a2a", a2ta, [128, E * 4 * 128])
            _a2b = load("a2b", a2tb, [128, E * 4 * 16])
            w_idm = load("idm", idm, [128, 128], f32)

            def _sl(tile_, idx, c):
                return tile_[:, idx * c:(idx + 1) * c]

            w_whh = lambda l, kt, ot: _sl(_whh, (l * KT + kt) * OT + ot, 128)
            w_wih = lambda l, kt, ot: _sl(_wih, (l * KT + kt) * OT + ot, 128)
            w_g0 = lambda kt, mt: _sl(_g0, kt * 4 + mt, 128)
            w_g1 = lambda kt, mt: _sl(_g1, kt * 4 + mt, 128)
            w_g2r = lambda kt: _sl(_g2r, kt, 4)
            w_a0 = lambda e, kt, mt: _sl(_a0, (e * 2 + kt) * 4 + mt, 128)
            w_a1 = lambda e, kt, mt: _sl(_a1, (e * 4 + kt) * 4 + mt, 128)
            w_a2a = lambda e, kt: _sl(_a2a, e * 4 + kt, 128)
            w_a2b = lambda e, kt: _sl(_a2b, e * 4 + kt, 16)

            ones1 = sp.tile([1, 128], bf16, tag="ones1")
            nc.gpsimd.memset(ones1[:], 1.0)
            zero_h = sp.tile([128, KT * B], bf16, tag="zero_h")
            nc.gpsimd.memset(zero_h[:], 0.0)

            # hidden-state history ring: rows = t % 32, layout (row, l, kt, b)
            # (layer 3's h lives in zfT instead; its hist slot stays unused).
            HROW = L * KT * B                               # 128 per row
            hist = sp.tile([128, 32 * HROW], bf16, tag="hist")
            zfT = sp.tile([128, KT * NTOK], bf16, tag="zfT")
            # gi buffer: (par, s, l, ot, b); written one wave ahead
            GROW = L * OT * B                               # 384 per step-row
            gi_all = sp.tile([128, 2 * CH * GROW], bf16, tag="gi_all")

            # layer-0 input chunks streamed from DRAM
            x_bufs = {}

            def load_x(c):
                t_ = xch.tile([128, KT * CH * B], bf16, tag="xc")
                nc.sync.dma_start(
                    t_[:].rearrange("p (k n) -> p k n", k=KT),
                    x_all_d[:].rearrange("p (k n) -> p k n", k=KT)[
                        :, :, c * CH * B:(c + 1) * CH * B])
                x_bufs[c] = t_

            # ---------------------------------------------- GRU helpers
            def h_src(l, t, kt):
                # bf16 hidden state [128, B] (step-matmul rhs)
                if t < 0:
                    return zero_h[:, kt * B:(kt + 1) * B]
                if l < 3:
                    r = (t % 32) * HROW
                    return hist[:, r + (l * KT + kt) * B: r + (l * KT + kt + 1) * B]
                return zfT[:, kt * NTOK + t * B: kt * NTOK + t * B + B]

            def h_prev3(l, t):
                # [128, kt, b] view of h(t)
                if t < 0:
                    return zero_h[:].rearrange("p (k b) -> p k b", b=B)
                if l < 3:
                    r = (t % 32) * HROW + l * KT * B
                    return hist[:, r:r + KT * B].rearrange(
                        "p (k b) -> p k b", b=B)
                return zfT[:].rearrange("p (k n) -> p k n", k=KT)[
                    :, :, t * B:(t + 1) * B]

            def gi3(l, t, o0, o1):
                # [128, o, b] gi slice for step t, o-tiles [o0, o1)
                par, s = (t // CH) % 2, t % CH
                base = (par * CH + s) * GROW + (l * OT + o0) * B
                return gi_all[:, base:base + (o1 - o0) * B].rearrange(
                    "p (o b) -> p o b", b=B)

            def gin_rhs(l, c, r0, r1, kt):
                # bulk-gi rhs rows [r0, r1) of chunk c of layer l's input
                if l == 0:
                    return x_bufs[c][:, kt * CH * B + r0 * B:
                                     kt * CH * B + r1 * B]
                s0 = (c * CH) % 32
                return hist[:].rearrange("p (s z) -> p s z", z=HROW)[
                    :, s0 + r0:s0 + r1,
                    ((l - 1) * KT + kt) * B:((l - 1) * KT + kt + 1) * B]

            # manually-sliced PSUM rings (bank-packed: 4 GRU step slots and
            # 2 bulk-gi slots share one bank each)
            HCH = CH // 2
            gh_ring = pgru.tile([128, 4 * OT * B], f32, tag="ghring")
            gib_ring = pgib.tile([128, 2 * HCH * B], f32, tag="gibring")
            rc = {"gh": 0, "gib": 0}

            # bulk gi: one (layer, chunk, ot, half) unit = 2 matmuls + 1 copy
            def emit_gi_unit(l, c, ot, half):
                par = c % 2
                r0 = half * HCH
                rc["gib"] += 1
                i = (rc["gib"] % 2) * HCH * B
                pb = gib_ring[:, i:i + HCH * B]
                for kt in range(KT):
                    nc.tensor.matmul(pb, w_wih(l, kt, ot),
                                     gin_rhs(l, c, r0, r0 + HCH, kt),
                                     start=(kt == 0), stop=(kt == KT - 1))
                dst = gi_all[:].rearrange("p (s z) -> p s z", z=GROW)[
                    :, par * CH + r0:par * CH + r0 + HCH,
                    (l * OT + ot) * B:(l * OT + ot + 1) * B]
                pb3 = pb.rearrange("p (s b) -> p s b", b=B)
                nc.vector.tensor_copy(dst, pb3)

            # one GRU step for layer l at absolute time t, split into stages
            # for stage-major (software-pipelined) emission across layers.
            # sigmoid is computed as tanh: ru' = tanh(x/2) = 2*sigmoid(x)-1;
            # host halves the whh n-rows so (ru'+1)*psum_n == sigmoid(r)*h_n.
            N_STAGES = 9

            def make_step(l, t):
                rc["gh"] += 1
                i0 = (rc["gh"] % 4) * OT * B
                pgs = lambda o0, o1: gh_ring[:, i0 + o0 * B:i0 + o1 * B]
                p3 = lambda a: a.rearrange("p (o b) -> p o b", b=B)
                st = {}

                def mm():
                    for ot in range(OT):
                        for kt in range(KT):
                            nc.tensor.matmul(
                                pgs(ot, ot + 1),
                                w_whh(l, kt, ot), h_src(l, t - 1, kt),
                                start=(kt == 0), stop=(kt == KT - 1))

                def add_rz():
                    st["s_in"] = wk.tile([128, 4 * B], f32, tag="s_in", name="s_in")
                    nc.vector.tensor_tensor(p3(st["s_in"][:]), p3(pgs(0, 4)),
                                            gi3(l, t, 0, 4), op=ALU.add)

                def act_rz():
                    st["ru"] = wk.tile([128, 4 * B], f32, tag="ru", name="ru")
                    nc.scalar.activation(st["ru"][:], st["s_in"][:], AF.Tanh,
                                         scale=0.5)

                def mul_hn():
                    st["hn"] = wk.tile([128, 2 * B], f32, tag="hn", name="hn")
                    nc.vector.scalar_tensor_tensor(
                        st["hn"][:], st["ru"][:, 0:2 * B], 1.0, pgs(4, 6),
                        op0=ALU.add, op1=ALU.mult)

                def add_n():
                    st["n_in"] = wk.tile([128, 2 * B], f32, tag="n_in", name="n_in")
                    nc.gpsimd.tensor_tensor(p3(st["n_in"][:]), p3(st["hn"][:]),
                                            gi3(l, t, 4, 6), op=ALU.add)

                def act_n():
                    st["nt"] = wk.tile([128, 2 * B], f32, tag="nt", name="nt")
                    nc.scalar.activation(st["nt"][:], st["n_in"][:], AF.Tanh)

                def sub_dt():
                    st["dt"] = wk.tile([128, 2 * B], f32, tag="dt", name="dt")
                    nc.gpsimd.tensor_tensor(p3(st["dt"][:]), h_prev3(l, t - 1),
                                            p3(st["nt"][:]), op=ALU.subtract)

                def mul_mt():
                    st["mt"] = wk.tile([128, 2 * B], f32, tag="mt", name="mt")
                    nc.vector.scalar_tensor_tensor(
                        st["mt"][:], st["ru"][:, 2 * B:4 * B], 1.0, st["dt"][:],
                        op0=ALU.add, op1=ALU.mult)

                def add_h():
                    nc.vector.scalar_tensor_tensor(
                        h_prev3(l, t), p3(st["mt"][:]), 0.5, p3(st["nt"][:]),
                        op0=ALU.mult, op1=ALU.add)

                return [mm, add_rz, act_rz, mul_hn, add_n, act_n, sub_dt,
                        mul_mt, add_h]

            # ---------------------------------------------- MoE generator
            def elu_emit(dst, src_ps, merge_eng):
                # dst(bf16) = elu(src) = (relu(x) - 1) + min(exp(x), 1)
                tr = we.tile([128, TOKC], f32, tag="elu_r")
                nc.vector.tensor_scalar(tr[:], src_ps, 0.0, -1.0,
                                        op0=ALU.max, op1=ALU.add)
                te = we.tile([128, TOKC], f32, tag="elu_e")
                nc.scalar.activation(te[:], src_ps, AF.Exp)
                nc.vector.scalar_tensor_tensor(dst, te[:], 1.0, tr[:],
                                               op0=ALU.min, op1=ALU.add)

            def moe_gen(ctk):
                def zc(kt):
                    return zfT[:, kt * NTOK + ctk * TOKC:
                               kt * NTOK + (ctk + 1) * TOKC]

                # gating layer 0
                g0o = mp.tile([128, 4 * TOKC], bf16, tag="g0o")
                for mt in range(4):
                    pb = pmo.tile([128, TOKC], f32, tag="mo")
                    for kt in range(KT):
                        nc.tensor.matmul(pb[:], w_g0(kt, mt), zc(kt),
                                         start=(kt == 0), stop=(kt == KT - 1))
                    elu_emit(g0o[:, mt * TOKC:(mt + 1) * TOKC], pb[:],
                             nc.gpsimd)
                    yield
                # gating layer 1
                g1o = mp.tile([128, 4 * TOKC], bf16, tag="g1o")
                for mt in range(4):
                    pb = pmo.tile([128, TOKC], f32, tag="mo")
                    for kt in range(4):
                        nc.tensor.matmul(pb[:], w_g1(kt, mt),
                                         g0o[:, kt * TOKC:(kt + 1) * TOKC],
                                         start=(kt == 0), stop=(kt == 3))
                    elu_emit(g1o[:, mt * TOKC:(mt + 1) * TOKC], pb[:],
                             nc.gpsimd)
                    yield
                # gating layer 2 (token-major) + softmax
                pg2 = psm.tile([128, 16], f32, tag="pg2")
                for tt in range(4):
                    for kt in range(4):
                        nc.tensor.matmul(
                            pg2[:, tt * 4:(tt + 1) * 4],
                            g1o[:, kt * TOKC + tt * 128:
                                kt * TOKC + (tt + 1) * 128],
                            w_g2r(kt), start=(kt == 0), stop=(kt == 3),
                            skip_group_check=True)
                yield
                g4 = lambda a: a.rearrange("p (g e) -> p g e", e=4)
                mx = wk.tile([128, 4], f32, tag="sm_mx")
                nc.vector.tensor_reduce(mx[:], g4(pg2[:]),
                                        axis=mybir.AxisListType.X, op=ALU.max)
                sub = wk.tile([128, 16], f32, tag="sm_sub")
                nc.vector.tensor_tensor(g4(sub[:]), g4(pg2[:]),
                                        mx[:].unsqueeze(2).broadcast_to(
                                            (128, 4, 4)), op=ALU.subtract)
                ex = wk.tile([128, 16], f32, tag="sm_ex")
                nc.scalar.activation(ex[:], sub[:], AF.Exp)
                sm = wk.tile([128, 4], f32, tag="sm_sum")
                nc.vector.tensor_reduce(sm[:], g4(ex[:]),
                                        axis=mybir.AxisListType.X, op=ALU.add)
                rec = wk.tile([128, 4], f32, tag="sm_rec")
                nc.vector.reciprocal(rec[:], sm[:])
                gw = wk.tile([128, 16], f32, tag="sm_gw")
                nc.vector.tensor_tensor(g4(gw[:]), g4(ex[:]),
                                        rec[:].unsqueeze(2).broadcast_to(
                                            (128, 4, 4)), op=ALU.mult)
                yield
                # transpose gw -> [4, 512], broadcast to [128, 512] per e
                pgt = psm.tile([4, TOKC], f32, tag="pgt")
                for tt in range(4):
                    nc.tensor.transpose(pgt[:, tt * 128:(tt + 1) * 128],
                                        gw[:, tt * 4:(tt + 1) * 4], w_idm[:])
                gwT = mp.tile([4, TOKC], bf16, tag="gwT")
                nc.vector.tensor_copy(gwT[:], pgt[:])
                gwf = mp.tile([1, E * TOKC], bf16, tag="gwf")
                nc.sync.dma_start(
                    gwf[:].rearrange("p (e n) -> p e n", e=E), gwT[:])
                gwb = mp.tile([128, E * TOKC], bf16, tag="gwb")
                for e in range(E):
                    pbc = pmo.tile([128, TOKC], f32, tag="mo")
                    nc.tensor.matmul(pbc[:], ones1[:],
                                     gwf[0:1, e * TOKC:(e + 1) * TOKC],
                                     start=True, stop=True)
                    nc.vector.tensor_copy(gwb[:, e * TOKC:(e + 1) * TOKC],
                                          pbc[:])
                    yield

                def gwb_e(e):
                    return gwb[:, e * TOKC:(e + 1) * TOKC]

                # blend 0: scale zf by gates (8 tiles), then mt-outer matmuls
                zsc = mp.tile([128, E * KT * TOKC], bf16, tag="zsc")
                for e in range(E):
                    for kt in range(KT):
                        nc.gpsimd.tensor_tensor(
                            _sl(zsc, e * KT + kt, TOKC), zc(kt), gwb_e(e),
                            op=ALU.mult)
                    yield
                h1o = mp.tile([128, 4 * TOKC], bf16, tag="h1o")
                for mt in range(4):
                    pb = pmo.tile([128, TOKC], f32, tag="mo")
                    for e in range(E):
                        for kt in range(KT):
                            nc.tensor.matmul(
                                pb[:], w_a0(e, kt, mt),
                                _sl(zsc, e * KT + kt, TOKC),
                                start=(e == 0 and kt == 0),
                                stop=(e == E - 1 and kt == KT - 1),
                                skip_group_check=True)
                    elu_emit(h1o[:, mt * TOKC:(mt + 1) * TOKC], pb[:],
                             nc.vector)
                    yield
                # blend 1: scale h1o (16 tiles), mt-outer
                h1sc = mp.tile([128, E * 4 * TOKC], bf16, tag="h1sc")
                for e in range(E):
                    for kt in range(4):
                        nc.gpsimd.tensor_tensor(
                            _sl(h1sc, e * 4 + kt, TOKC),
                            h1o[:, kt * TOKC:(kt + 1) * TOKC], gwb_e(e),
                            op=ALU.mult)
                    yield
                h2o = mp.tile([128, 4 * TOKC], bf16, tag="h2o")
                for mt in range(4):
                    pb = pmo.tile([128, TOKC], f32, tag="mo")
                    for e in range(E):
                        for kt in range(4):
                            nc.tensor.matmul(
                                pb[:], w_a1(e, kt, mt),
                                _sl(h1sc, e * 4 + kt, TOKC),
                                start=(e == 0 and kt == 0),
                                stop=(e == E - 1 and kt == 3),
                                skip_group_check=True)
                    elu_emit(h2o[:, mt * TOKC:(mt + 1) * TOKC], pb[:],
                             nc.vector)
                    yield
                # blend 2: out 144 = 128 + 16 (e-outer, scaled input per e)
                poa = pmo.tile([128, TOKC], f32, tag="mo")
                pob = psm.tile([16, TOKC], f32, tag="pob")
                for e in range(E):
                    h2sc = msc.tile([128, 4 * TOKC], bf16, tag="h2sc")
                    for kt in range(4):
                        nc.gpsimd.tensor_tensor(
                            _sl(h2sc, kt, TOKC),
                            h2o[:, kt * TOKC:(kt + 1) * TOKC], gwb_e(e),
                            op=ALU.mult)
                    yield
                    for kt in range(4):
                        last = (e == E - 1 and kt == 3)
                        nc.tensor.matmul(poa[:], w_a2a(e, kt),
                                         _sl(h2sc, kt, TOKC),
                                         start=(e == 0 and kt == 0),
                                         stop=last, skip_group_check=True)
                        nc.tensor.matmul(pob[:], w_a2b(e, kt),
                                         _sl(h2sc, kt, TOKC),
                                         start=(e == 0 and kt == 0),
                                         stop=last, skip_group_check=True)
                    yield
                oa = mp.tile([128, TOKC], f32, tag="oa")
                nc.vector.tensor_copy(oa[:], poa[:])
                ob = mp.tile([16, TOKC], f32, tag="ob")
                nc.vector.tensor_copy(ob[:], pob[:])
                t0 = ctk * (TOKC // B)
                nc.sync.dma_start(out_d[0:128, t0:t0 + 32, :],
                                  oa[:].rearrange("p (t b) -> p t b", b=B))
                nc.sync.dma_start(out_d[128:144, t0:t0 + 32, :],
                                  ob[:].rearrange("p (t b) -> p t b", b=B))
                yield

            # ---------------------------------------------- main schedule
            NW = NCH + L - 1
            moe_ready = {2 * c + 5: c for c in range(NMC)}
            moe_active = []

            def drain_moe(n):
                while n > 0 and moe_active:
                    try:
                        next(moe_active[0])
                        n -= 1
                    except StopIteration:
                        moe_active.pop(0)

            load_x(0)                 # layer-0 chunk 0 input + gi before wave 0
            for ot in range(OT):
                for hf in range(2):
                    emit_gi_unit(0, 0, ot, hf)
            for w in range(NW):
                active = [l for l in range(L) if 0 <= w - l < NCH]
                nxt = [l for l in range(L) if 0 <= w + 1 - l < NCH]
                if w in moe_ready:
                    moe_active.append(moe_gen(moe_ready[w]))
                gi_early = ([(0, w + 1, ot, hf) for ot in range(OT)
                             for hf in range(2)] if (0 in nxt and w + 1 < NCH)
                            else [])
                if gi_early:
                    load_x(w + 1)
                gi_h0 = [(l, w + 1 - l, ot, 0) for l in nxt if l >= 1
                         for ot in range(OT)]
                gi_h1 = [(l, w + 1 - l, ot, 1) for l in nxt if l >= 1
                         for ot in range(OT)]
                for s in range(CH):
                    stages = [make_step(l, (w - l) * CH + s) for l in active]
                    for si in range(N_STAGES):
                        for st in stages:
                            st[si]()
                    if s < 6:
                        for u in gi_early[2 * s:2 * s + 2]:
                            emit_gi_unit(*u)
                    if s >= 8:
                        k = (s - 8) * 3
                        for u in gi_h0[k:k + 3]:
                            emit_gi_unit(*u)
                    drain_moe(2)
                for u in gi_h1:
                    emit_gi_unit(*u)
            for w_, c_ in sorted(moe_ready.items()):
                if w_ >= NW:          # chunks whose ready-wave is past the end
                    moe_active.append(moe_gen(c_))
            while moe_active:
                drain_moe(1000)

            if debug:
                nc.sync.dma_start(dbg["dbg_zf"][:], zfT[:])
                nc.sync.dma_start(dbg["dbg_gi"][:], gi_all[:])
    return nc


# ------------------------------------------------------------- walrus fixup
def _fix_sync_waits(nc, max_waits=1):
    """This walrus build allows only one sync wait per instruction; move
    excess waits onto NOPs inserted ahead of the instruction."""
    import concourse.mybir as mybir
    import bass_rust
    ctr = 0
    for f in nc.m.functions:
        for blk in f.blocks:
            out = []
            changed = False
            for inst in blk.instructions:
                si = inst.sync_info
                if si is not None and si.on_wait and len(si.on_wait) > max_waits:
                    waits = list(si.on_wait)
                    extra, keep = waits[:-max_waits], waits[-max_waits:]
                    for w_ in extra:
                        ctr += 1
                        nop = mybir.InstNoOp(name=f"WSPLIT-{ctr}", ins=[], outs=[])
                        nop.engine = inst.engine
                        nop.sync_info = bass_rust.SyncInfo(on_wait=[w_], on_update=[])
                        out.append(nop)
                    inst.sync_info = bass_rust.SyncInfo(
                        on_wait=keep, on_update=list(si.on_update))
                    changed = True
                out.append(inst)
            if changed:
                blk.instructions = out
    return ctr


# ------------------------------------------------------------- preprocessing
def _prep_core_inputs(inputs, T_=T):
    z = _f32(inputs["z"])
    y = np.asarray(inputs["y"]).astype(np.int64)
    lengths = np.asarray(inputs["lengths"]).astype(np.float64)
    emb_w = _f32(inputs["emb_w"])      # [H, D+NC+1]
    gru_wih = _f32(inputs["gru_wih"])  # [4, 3H, H]
    gru_whh = _f32(inputs["gru_whh"])
    g0_w = _f32(inputs["g0_w"]); g1_w = _f32(inputs["g1_w"]); g2_w = _f32(inputs["g2_w"])
    g0_b = _f32(inputs["g0_b"]); g1_b = _f32(inputs["g1_b"]); g2_b = _f32(inputs["g2_b"])
    a0 = _f32(inputs["alpha0"]); a1 = _f32(inputs["alpha1"]); a2 = _f32(inputs["alpha2"])
    b0 = _f32(inputs["beta0"]); b1 = _f32(inputs["beta1"]); b2 = _f32(inputs["beta2"])
    emb_b = _f32(inputs["emb_b"])
    bsum = _f32(inputs["gru_bih"]) + _f32(inputs["gru_bhh"])  # [4, 3H] assumed zero

    def pack_lhsT(w, cols=128):
        # w: [O, K]; lhsT = w.T tiled [K//128, O//cols, 128, cols]
        # -> flat [128, ntiles*cols], tile index = kt*OT_ + ot (kt-major)
        O, K = w.shape
        ktn, otn = K // 128, O // cols
        wt = np.ascontiguousarray(w.T).reshape(ktn, 128, otn, cols)
        return wt.transpose(1, 0, 2, 3).reshape(128, ktn * otn * cols)

    # halve the n-gate rows of whh: the kernel computes sigmoid via
    # ru' = tanh(x/2) and then sigmoid(r)*h_n as (ru'+1) * (0.5*h_n).
    whh_sc = gru_whh.copy()
    whh_sc[:, 2 * H:, :] *= 0.5
    whh_t = _bf16(np.concatenate([pack_lhsT(whh_sc[l]) for l in range(4)], axis=1))
    wih_t = _bf16(np.concatenate([pack_lhsT(gru_wih[l]) for l in range(4)], axis=1))
    g0t = _bf16(pack_lhsT(g0_w))
    g1t = _bf16(pack_lhsT(g1_w))
    g2r = _bf16(np.ascontiguousarray(g2_w.T).reshape(4, 128, 4)
                .transpose(1, 0, 2).reshape(128, 16))
    a0t = _bf16(np.concatenate([pack_lhsT(a0[e]) for e in range(E)], axis=1))
    a1t = _bf16(np.concatenate([pack_lhsT(a1[e]) for e in range(E)], axis=1))
    a2T = np.stack([np.ascontiguousarray(a2[e].T) for e in range(E)])  # [E,512,144]
    a2r = a2T.reshape(E, 4, 128, 144)
    a2ta = _bf16(a2r[:, :, :, 0:128].transpose(2, 0, 1, 3).reshape(128, E * 4 * 128))
    a2tb = _bf16(a2r[:, :, :, 128:144].transpose(2, 0, 1, 3).reshape(128, E * 4 * 16))
    idm = _f32(np.eye(128))
    for _nm, _v in [("g0_b", g0_b), ("g1_b", g1_b), ("g2_b", g2_b),
                    ("emb_b", emb_b), ("beta0", b0), ("beta1", b1),
                    ("beta2", b2), ("gru_b", bsum)]:
        assert np.abs(_v).max() == 0.0, f"{_nm} nonzero; kernel assumes zero"

    # ---- per-core layer-0 input sequence x_all[f, (kt|t|b)]
    yoh = np.zeros((BS, NCLS), np.float32)
    yoh[np.arange(BS), y] = 1.0
    za = np.concatenate([z, yoh], axis=1)              # [BS, 268]
    u_all = za @ emb_w[:, :268].T                      # [BS, H]
    cb = (1.0 / (lengths - 1.0)).astype(np.float32)    # time scale per row
    e268 = emb_w[:, 268]                               # [H]
    tt_ = np.arange(T_, dtype=np.float32)

    maps = []
    for cidx in range(NCORES):
        sl = slice(cidx * B, (cidx + 1) * B)
        # x[f, t, b] = u[b, f] + t * cb_b * e268[f]
        xc = (u_all[sl].T[:, None, :]
              + e268[:, None, None] * (tt_[None, :, None] * cb[sl][None, None, :]))
        x_pack = _bf16(xc.reshape(KT, 128, T_ * B)
                       .transpose(1, 0, 2).reshape(128, KT * T_ * B))
        maps.append(dict(
            whh_t=whh_t, wih_t=wih_t, x_all=x_pack,
            g0t=g0t, g1t=g1t, g2r=g2r,
            a0t=a0t, a1t=a1t, a2ta=a2ta, a2tb=a2tb, idm=idm,
        ))
    return maps


# ------------------------------------------------------------------ runner
def _get_runner(T_=T):
    key = ("runner", T_)
    if key in _STATE:
        return _STATE[key]
    import jax
    from jax.sharding import Mesh, PartitionSpec
    try:
        from jax.experimental.shard_map import shard_map
    except ImportError:
        from jax.shard_map import shard_map
    import concourse.mybir as mybir
    from concourse import bass2jax

    nc = _build_nc(T_)
    _fix_sync_waits(nc)
    bass2jax.install_neuronx_cc_hook()
    partition_name = nc.partition_id_tensor.name if nc.partition_id_tensor else None
    in_names, out_names, out_avals = [], [], []
    for alloc in nc.m.functions[0].allocations:
        if not isinstance(alloc, mybir.MemoryLocationSet):
            continue
        name = alloc.memorylocations[0].name
        if alloc.kind == "ExternalInput":
            if name != partition_name:
                in_names.append(name)
        elif alloc.kind == "ExternalOutput":
            out_names.append(name)
            out_avals.append(jax.core.ShapedArray(
                tuple(alloc.tensor_shape), mybir.dt.np(alloc.dtype)))
    n_params = len(in_names)
    all_names = in_names + out_names + ([partition_name] if partition_name else [])

    def _body(*args):
        operands = list(args)
        if partition_name is not None:
            operands.append(bass2jax.partition_id_tensor())
        return tuple(bass2jax._bass_exec_p.bind(
            *operands, out_avals=tuple(out_avals), in_names=tuple(all_names),
            out_names=tuple(out_names), lowering_input_output_aliases=(),
            sim_require_finite=True, sim_require_nnan=True, nc=nc))

    devices = jax.devices()[:NCORES]
    mesh = Mesh(np.asarray(devices), ("core",))
    n_outs = len(out_names)
    sharded = jax.jit(
        shard_map(_body, mesh=mesh,
                  in_specs=(PartitionSpec("core"),) * (n_params + n_outs),
                  out_specs=(PartitionSpec("core"),) * n_outs),
        keep_unused=True)
    runner = dict(sharded=sharded, in_names=in_names, out_names=out_names,
                  out_avals=out_avals)
    _STATE[key] = runner
    return runner


def _run_device(maps, T_=T, timing=None):
    import jax
    r = _get_runner(T_)
    concat_in = [np.concatenate([np.asarray(maps[c][n]) for c in range(NCORES)],
                                axis=0) for n in r["in_names"]]
    zeros = [np.zeros((NCORES * a.shape[0], *a.shape[1:]), a.dtype)
             for a in r["out_avals"]]
    din = [jax.device_put(x) for x in concat_in]
    dz = [jax.device_put(z) for z in zeros]
    out = r["sharded"](*din, *dz)
    jax.block_until_ready(out)
    if timing is not None:
        import time
        for _ in range(timing.get("iters", 10)):
            t0 = time.perf_counter()
            out = r["sharded"](*din, *dz)
            jax.block_until_ready(out)
            timing.setdefault("times", []).append(time.perf_counter() - t0)
    o = np.asarray(out[0])
    per_core = o.reshape(NCORES, OUTD, T_, B)
    return per_core


def kernel(**inputs):
    maps = _prep_core_inputs(inputs, T)
    per_core = _run_device(maps, T)          # [NCORES, OUTD, T, B]
    full = per_core.transpose(0, 3, 1, 2)    # [NCORES, B, OUTD, T]
    full = full.reshape(BS, NJ, NF, T)
    return np.ascontiguousarray(full.astype(np.float32))
